# revision 4
# baseline (speedup 1.0000x reference)
"""Trainium2 Bass kernel for GroupingUnit2D (vq_codebook).

Reference computation (see problem):
    x = node_feats.reshape(-1, C)                       # [N, C], N = S*G*B
    logits = min(2*x@W.T - |x|^2 - |c|^2, 0) / sigmoid(smooth)
    assign = softmax(logits, axis=1)                    # [N, P]
    out[g] = sum_{s,b} assign[s,g,b]^T @ x[s,g,b]       # [G, P, C]

Sharding: one graph per NeuronCore (G == n_cores == 8).  Each core handles
S*B = 6144 nodes and produces its graph's full [P, C] output plus its slice
of `assign` -- no collectives needed, outputs are gathered on the host.

Precision: the softmax is effectively one-hot (logits are O(-1000) with
top-2 gaps that can be < 1), so the score matmul runs as a 3-term hi/lo
bf16 split (x_hi@w_hi + x_hi@w_lo + x_lo@w_hi) accumulated in fp32 PSUM,
which matches fp32 to ~2e-4.  The grouped-output matmul runs in bf16.
|x|^2 is computed on the host in fp64->fp32 (exact), folded into the
logit affine together with 1/beta and |c|^2/beta.
"""

import numpy as np
import ml_dtypes

import concourse.bass as bass
import concourse.mybir as mybir
from concourse import tile
from concourse.vector_clock import ScopedClock, VectorClock
from concourse.tile_sem_assignment import N_PROCS
from concourse.bass_utils import run_bass_kernel_spmd

BF16 = ml_dtypes.bfloat16
F32 = mybir.dt.float32
BF = mybir.dt.bfloat16

S, G, B, C, P = 3, 8, 2048, 512, 64
NODES = S * B            # 6144 nodes per core
NCH = NODES // 128       # 48 node chunks of 128
KCH = C // 128           # 4 contraction chunks of 128


class _TC(tile.TileContext):
    """TileContext whose tail drain splits its semaphore waits.

    The walrus build in this container rejects instructions carrying more
    than one sync-wait ("Too many sync wait commands"), and the stock
    _drain_and_barrier attaches one wait per live logical proc to a single
    Drain.  Emit one Drain per proc instead.
    """

    def _drain_and_barrier(self, tick_clock, wait_clock):
        g = tick_clock.global_clock
        for p in range(N_PROCS):
            if g[p] == 0:
                continue
            single = VectorClock([g[q] if q == p else 0 for q in range(N_PROCS)])
            di = self.nc.sync.drain()
            wait_clock.add_sem_waits(di.ins, ScopedClock({None: single}))
        self.nc.all_engine_barrier()
        assert self.sems is not None
        popped = self.nc._tile_sem_poison_stack.pop()
        assert popped is self._sem_poison
        self.nc.clear_and_free_semaphores(list(self.sems.allocated().values()))
        self.nc.all_engine_barrier()


def _split_multi_waits(nc: bass.Bass) -> None:
    """walrus here allows at most one sync-wait per instruction; hoist the
    extras onto same-engine NoOps inserted just before the instruction."""
    for fn in nc.m.functions:
        for blk in fn.blocks:
            insts = blk.instructions
            for idx in range(len(insts) - 1, -1, -1):
                inst = insts[idx]
                si = inst.sync_info
                if si is None or len(si.on_wait) <= 1:
                    continue
                waits = list(si.on_wait)
                inst.sync_info = mybir.SyncInfo(
                    on_wait=[waits[-1]], on_update=list(si.on_update)
                )
                for w in reversed(waits[:-1]):
                    nop = mybir.InstNoOp(
                        name=nc.get_next_instruction_name(),
                        engine=inst.engine,
                        bass_nofuse=True,
                        sync_info=mybir.SyncInfo(on_wait=[w], on_update=[]),
                    )
                    nc.register_instruction(nop)
                    insts.insert(idx, nop)


def build_program() -> bass.Bass:
    nc = bass.Bass()

    xhi_d = nc.dram_tensor("xhi", [NCH, 128, C], mybir.dt.bfloat16, kind="ExternalInput")
    xthi_d = nc.dram_tensor("xthi", [NCH, KCH, 128, 128], mybir.dt.bfloat16, kind="ExternalInput")
    xtlo_d = nc.dram_tensor("xtlo", [NCH, KCH, 128, 128], mybir.dt.bfloat16, kind="ExternalInput")
    xsq_d = nc.dram_tensor("xsq", [128, NCH], F32, kind="ExternalInput")
    wcomb_d = nc.dram_tensor("wcomb", [KCH, 128, 128], mybir.dt.bfloat16, kind="ExternalInput")
    invb_d = nc.dram_tensor("invb", [128, P], F32, kind="ExternalInput")
    db_d = nc.dram_tensor("db", [128, P], F32, kind="ExternalInput")

    assign_d = nc.dram_tensor("assign", [NCH, 128, P], F32, kind="ExternalOutput")
    gout_d = nc.dram_tensor("gout", [P, C], F32, kind="ExternalOutput")

    with _TC(nc) as tc:
        with (
            tc.tile_pool(name="const", bufs=1) as cpool,
            tc.tile_pool(name="xin", bufs=3) as xpool,
            tc.tile_pool(name="mid", bufs=3) as mpool,
            tc.tile_pool(name="cxps", bufs=2, space="PSUM") as cxpool,
            tc.tile_pool(name="outps", bufs=1, space="PSUM") as opool,
        ):
            wcomb_sb = cpool.tile([128, KCH * 128], BF, tag="wcomb")
            for k in range(KCH):
                nc.sync.dma_start(out=wcomb_sb[:, k * 128:(k + 1) * 128], in_=wcomb_d[k])
            invb_sb = cpool.tile([128, P], F32, tag="invb")
            nc.sync.dma_start(out=invb_sb[:], in_=invb_d[:])
            db_sb = cpool.tile([128, P], F32, tag="db")
            nc.sync.dma_start(out=db_sb[:], in_=db_d[:])
            xsq_sb = cpool.tile([128, NCH], F32, tag="xsq")
            nc.sync.dma_start(out=xsq_sb[:], in_=xsq_d[:])

            gout_ps = opool.tile([P, C], F32, tag="gout")

            for j in range(NCH):
                xhi_t = xpool.tile([128, C], BF, tag="xhi")
                nc.sync.dma_start(out=xhi_t[:], in_=xhi_d[j])
                xthi_t = xpool.tile([128, KCH * 128], BF, tag="xthi")
                xtlo_t = xpool.tile([128, KCH * 128], BF, tag="xtlo")
                for k in range(KCH):
                    nc.sync.dma_start(out=xthi_t[:, k * 128:(k + 1) * 128], in_=xthi_d[j, k])
                    nc.sync.dma_start(out=xtlo_t[:, k * 128:(k + 1) * 128], in_=xtlo_d[j, k])

                # cx psum: cols 0:64 accumulate x_hi@w_hi + x_lo@w_hi,
                #          cols 64:128 accumulate x_hi@w_lo
                ps = cxpool.tile([128, 128], F32, tag="cx")
                for k in range(KCH):
                    nc.tensor.matmul(
                        ps[:],
                        lhsT=xthi_t[:, k * 128:(k + 1) * 128],
                        rhs=wcomb_sb[:, k * 128:(k + 1) * 128],
                        start=(k == 0), stop=False,
                    )
                for k in range(KCH):
                    nc.tensor.matmul(
                        ps[:, 0:P],
                        lhsT=xtlo_t[:, k * 128:(k + 1) * 128],
                        rhs=wcomb_sb[:, k * 128:k * 128 + P],
                        start=False, stop=(k == KCH - 1),
                    )

                # t = cx_hi + cx_lo - xsq*invb - db ; m = rowmax(t)
                u_t = mpool.tile([128, P], F32, tag="u")
                nc.scalar.activation(
                    u_t[:], invb_sb[:], mybir.ActivationFunctionType.Copy,
                    scale=xsq_sb[:, j:j + 1],
                )
                v_t = mpool.tile([128, P], F32, tag="v")
                nc.vector.tensor_tensor(v_t[:], u_t[:], db_sb[:], mybir.AluOpType.add)
                vp_t = mpool.tile([128, P], F32, tag="vp")
                nc.vector.tensor_tensor(vp_t[:], v_t[:], ps[:, P:2 * P], mybir.AluOpType.subtract)
                t_t = mpool.tile([128, P], F32, tag="t")
                nc.vector.tensor_tensor(
                    t_t[:], ps[:, 0:P], vp_t[:], mybir.AluOpType.subtract)
                m_t = mpool.tile([128, 1], F32, tag="m")
                nc.vector.tensor_reduce(
                    m_t[:], t_t[:], mybir.AxisListType.X, mybir.AluOpType.max)
                negm_t = mpool.tile([128, 1], F32, tag="negm")
                nc.vector.tensor_scalar_mul(negm_t[:], m_t[:], -1.0)

                e_t = mpool.tile([128, P], F32, tag="e")
                ssum_t = mpool.tile([128, 1], F32, tag="ssum")
                nc.scalar.activation(
                    e_t[:], t_t[:], mybir.ActivationFunctionType.Exp,
                    bias=negm_t[:], scale=1.0, accum_out=ssum_t[:],
                )
                r_t = mpool.tile([128, 1], F32, tag="r")
                nc.vector.reciprocal(r_t[:], ssum_t[:])

                a32_t = mpool.tile([128, P], F32, tag="a32")
                nc.scalar.activation(
                    a32_t[:], e_t[:], mybir.ActivationFunctionType.Copy,
                    scale=r_t[:],
                )
                a16_t = mpool.tile([128, P], BF, tag="a16")
                nc.vector.tensor_scalar_mul(a16_t[:], e_t[:], r_t[:])

                nc.sync.dma_start(out=assign_d[j], in_=a32_t[:])

                nc.tensor.matmul(
                    gout_ps[:], lhsT=a16_t[:], rhs=xhi_t[:],
                    start=(j == 0), stop=(j == NCH - 1),
                )

            gout_sb = cpool.tile([P, C], F32, tag="goutsb")
            nc.vector.tensor_copy(gout_sb[:], gout_ps[:])
            nc.sync.dma_start(out=gout_d[:], in_=gout_sb[:])

    _split_multi_waits(nc)
    return nc


_NC_CACHE = None


def _get_program():
    global _NC_CACHE
    if _NC_CACHE is None:
        _NC_CACHE = build_program()
    return _NC_CACHE


def make_inmaps(node_feats, weight, smooth_factor):
    """Host-side shard + layout prep.  Returns list of 8 per-core input dicts."""
    x = np.ascontiguousarray(np.asarray(node_feats, dtype=np.float32)).reshape(S, G, B, C)
    w = np.asarray(weight, dtype=np.float32)
    sm = np.asarray(smooth_factor, dtype=np.float32)

    beta = 1.0 / (1.0 + np.exp(-sm.astype(np.float64)))
    invb = (1.0 / beta).astype(np.float32)                      # [P]
    c_sq = np.sum(w.astype(np.float64) ** 2, axis=1)
    d = (c_sq / beta).astype(np.float32)                        # [P]

    w2b = (2.0 * w * invb[:, None]).astype(np.float32)          # [P, C]
    whi = w2b.astype(BF16)
    wlo = (w2b - whi.astype(np.float32)).astype(BF16)
    # wcomb[k, ci, 0:64] = whi.T chunk, [k, ci, 64:128] = wlo.T chunk
    wcomb = np.concatenate(
        [np.ascontiguousarray(whi.T).reshape(KCH, 128, P),
         np.ascontiguousarray(wlo.T).reshape(KCH, 128, P)], axis=2,
    )
    wcomb = np.ascontiguousarray(wcomb)

    invb_bc = np.ascontiguousarray(np.broadcast_to(invb[None, :], (128, P)))
    db_bc = np.ascontiguousarray(np.broadcast_to(d[None, :], (128, P)))

    in_maps = []
    for g in range(G):
        xg = np.ascontiguousarray(x[:, g]).reshape(NODES, C)    # [6144, 512] f32
        xhi = xg.astype(BF16)
        xlo = (xg - xhi.astype(np.float32)).astype(BF16)
        # [j, k, ci, ni] = x[j*128+ni, k*128+ci]
        xthi = np.ascontiguousarray(xhi.reshape(NCH, 128, KCH, 128).transpose(0, 2, 3, 1))
        xtlo = np.ascontiguousarray(xlo.reshape(NCH, 128, KCH, 128).transpose(0, 2, 3, 1))
        xsq = np.sum(xg.astype(np.float64) ** 2, axis=1).astype(np.float32)
        xsq_t = np.ascontiguousarray(xsq.reshape(NCH, 128).T)   # [128, 48]
        in_maps.append({
            "xhi": np.ascontiguousarray(xhi.reshape(NCH, 128, C)),
            "xthi": xthi,
            "xtlo": xtlo,
            "xsq": xsq_t,
            "wcomb": wcomb,
            "invb": invb_bc,
            "db": db_bc,
        })
    return in_maps


def kernel(node_feats, weight, smooth_factor, batch_size, num_graphs):
    assert int(batch_size) == B and int(num_graphs) == G
    nf = np.asarray(node_feats, dtype=np.float32)
    assert nf.shape == (S, G * B, C), nf.shape

    nc = _get_program()
    in_maps = make_inmaps(nf, weight, smooth_factor)
    res = run_bass_kernel_spmd(nc, in_maps, list(range(G)))

    outputs = np.empty((G, P, C), np.float32)
    assign = np.empty((S, G, B, P), np.float32)
    for g in range(G):
        outputs[g] = res.results[g]["gout"]
        assign[:, g] = res.results[g]["assign"].reshape(S, B, P)
    return outputs, assign.reshape(S * G * B, P)
